# revision 16
# baseline (speedup 1.0000x reference)
"""Trainium2 Bass kernel for nn_Consolidation_111669149787.

Computation (reference):
  mxt = transpose(mx, (2,1,0,3))          # [N,B,T,D] -> [T,B,N,D]
  h   = mxt @ W.T                          # einsum tbnd,ed->tbne
  hn  = BatchNorm1d over (T*B*N) per channel e (biased var, training mode)
  s   = MultiStepLIF(hn): v <- v + (x-v)/2; s = H(v-0.5); v <- v*(1-s)
  out = x * s

Sharding: data-parallel over B (8 cores x B_loc=8); W/gamma/beta replicated.
BN stats are all-reduced across cores (1KB collective).

Key kernel trick: fold BN into the LIF recurrence. With A = gamma*invstd,
C = beta - mean*A (so hn = A*h + C) and state z = v/A - C/A, the recursion
becomes stats-free:  z_t = z_{t-1}/2 + h_t/2 ,  spike s = H(z - phi) with
phi = (0.5 - C)/A + ... (phi = (0.5-beta)/A + mean), reset z <- -C/A = negk.
So phase 1 (matmul + stat partials) runs before stats are known, and phase 2
is 3 DVE ops + 1 GpSimd op per [128,512] tile-step.
"""

import sys

sys.path.insert(0, "/opt/trn_rl_repo")

import numpy as np

# Full-problem constants (hardcoded per contract: kernel.py is self-contained)
T_FULL, B_FULL, N_FULL, D_FULL = 4, 64, 256, 512
N_CORES = 8
TAU_HALF = 0.5  # 1/tau with tau=2.0
V_TH = 0.5
BN_EPS = 1e-5

_BUILD_CACHE = {}


def build_kernel(T, B_loc, N, D, n_cores, r_tot, sim=False):
    """Build the SPMD Bass program for one core's shard.

    Per-core inputs: mx [N, B_loc, T, D], x [T, B_loc, N, D], W [D, D],
    gamma [1, D], beta [1, D].  Output: out [T, B_loc, N, D].
    r_tot = T * (B_loc*n_cores) * N  (global BN sample count).
    """
    import concourse.bass as bass
    import concourse.bacc as bacc
    import concourse.mybir as mybir
    import concourse.tile as tile
    from concourse import masks

    fp32 = mybir.dt.float32
    P = 128
    assert D % P == 0 and N % P == 0
    DB = D // P  # d-blocks (4)
    NT = N // P  # n-tiles per (t, b) (2)
    n_tiles = T * B_loc * NT  # 64 phase-1 tiles per core
    ALU = mybir.AluOpType
    AF = mybir.ActivationFunctionType

    if sim:
        nc = bacc.Bacc(None, num_devices=n_cores, target_bir_lowering=False, debug=True)
    else:
        nc = bacc.Bacc(None, num_devices=n_cores)

    mx_d = nc.declare_dram_parameter("mx", [N, B_loc, T, D], fp32, isOutput=False)
    x_d = nc.declare_dram_parameter("x", [T, B_loc, N, D], fp32, isOutput=False)
    w_d = nc.declare_dram_parameter("W", [D, D], fp32, isOutput=False)
    gamma_d = nc.declare_dram_parameter("gamma", [1, D], fp32, isOutput=False)
    beta_d = nc.declare_dram_parameter("beta", [1, D], fp32, isOutput=False)
    out_d = nc.declare_dram_parameter("out", [T, B_loc, N, D], fp32, isOutput=True)

    with tile.TileContext(nc) as tc:
        with (
            tc.tile_pool(name="persist", bufs=1) as persist,
            tc.tile_pool(name="mx_in", bufs=3) as mx_pool,
            tc.tile_pool(name="mxT", bufs=2) as mxT_pool,
            tc.tile_pool(name="sq", bufs=2) as sq_pool,
            tc.tile_pool(name="x_in", bufs=3) as x_pool,
            tc.tile_pool(name="out_st", bufs=3) as out_pool,
            tc.tile_pool(name="spk", bufs=2) as s_pool,
            tc.tile_pool(name="zst", bufs=2) as z_pool,
            tc.tile_pool(name="small", bufs=1) as small,
            tc.tile_pool(name="ps_tr", bufs=2, space="PSUM") as ps_tr,
            tc.tile_pool(name="ps_h", bufs=2, space="PSUM") as ps_h,
            tc.tile_pool(name="ps_misc", bufs=1, space="PSUM") as ps_misc,
            tc.tile_pool(name="dram", bufs=2, space="DRAM") as dram,
        ):
            # ---------- setup ----------
            identity = persist.tile([P, P], fp32)
            masks.make_identity(nc, identity[:])
            ones_col = persist.tile([P, 1], fp32)  # lhsT for partition reduce
            nc.gpsimd.memset(ones_col[:], 1.0)
            ones_row = persist.tile([1, P], fp32)  # lhsT for partition broadcast
            nc.gpsimd.memset(ones_row[:], 1.0)

            gamma_sb = small.tile([1, D], fp32)
            beta_sb = small.tile([1, D], fp32)
            nc.sync.dma_start(gamma_sb[:], gamma_d[:])
            nc.sync.dma_start(beta_sb[:], beta_d[:])

            # W [e, d] natural -> WT[db] tiles [128 d-in-block, D e]
            WT = [
                persist.tile([P, D], fp32, name=f"WT{db}", tag=f"WT{db}")
                for db in range(DB)
            ]
            for eb in range(DB):
                w_nat = mx_pool.tile([P, D], fp32, tag="mx_in")
                nc.sync.dma_start(w_nat[:], w_d[eb * P : (eb + 1) * P, :])
                wt_ps = ps_tr.tile([P, P], fp32, tag="wtps", bufs=1)
                for db in range(DB):
                    nc.tensor.transpose(
                        wt_ps[:], w_nat[:, db * P : (db + 1) * P], identity[:]
                    )
                    nc.scalar.copy(WT[db][:, eb * P : (eb + 1) * P], wt_ps[:])

            # stat partial accumulators
            S_acc = persist.tile([P, D], fp32)  # sum of h/2
            Q_acc = persist.tile([P, D], fp32)  # sum of h^2
            nc.gpsimd.memset(S_acc[:], 0.0)
            nc.vector.memset(Q_acc[:], 0.0)

            # big h/2 store: one [128, n_tiles*D] fp32 tile
            h2_store = persist.tile([P, n_tiles * D], fp32)

            # ---------- phase 1: matmul + stat partials ----------
            for t in range(T):
                for b in range(B_loc):
                    for ntl in range(NT):
                        tid = (t * B_loc + b) * NT + ntl
                        n0 = ntl * P
                        mx_in = mx_pool.tile([P, D], fp32)
                        nc.sync.dma_start(
                            mx_in[:], mx_d[n0 : n0 + P, b, t, :]
                        )
                        tr_ps = ps_tr.tile([P, D], fp32, tag="trps")
                        for db in range(DB):
                            nc.tensor.transpose(
                                tr_ps[:, db * P : (db + 1) * P],
                                mx_in[:, db * P : (db + 1) * P],
                                identity[:],
                            )
                        mxT = mxT_pool.tile([P, D], fp32)
                        nc.scalar.copy(mxT[:], tr_ps[:])
                        h_ps = ps_h.tile([P, D], fp32)
                        for db in range(DB):
                            nc.tensor.matmul(
                                h_ps[:],
                                mxT[:, db * P : (db + 1) * P],
                                WT[db][:],
                                start=(db == 0),
                                stop=(db == DB - 1),
                            )
                        h2 = h2_store[:, tid * D : (tid + 1) * D]
                        # h2 = h/2 (the LIF recursion consumes h/2 directly)
                        nc.scalar.activation(h2, h_ps[:], AF.Copy, scale=0.5)
                        sq = sq_pool.tile([P, D], fp32)
                        nc.scalar.activation(sq, h_ps[:], AF.Square)
                        nc.gpsimd.tensor_tensor(S_acc[:], S_acc[:], h2, ALU.add)
                        nc.vector.tensor_tensor(Q_acc[:], Q_acc[:], sq[:], ALU.add)

            # ---------- phase 1.5: stats + collective + fold ----------
            red_ps_s = ps_misc.tile([1, D], fp32, name="red_ps_s", tag="redps")
            red_ps_q = ps_misc.tile([1, D], fp32, name="red_ps_q", tag="redps")
            nc.tensor.matmul(red_ps_s[:], ones_col[:], S_acc[:], start=True, stop=True)
            nc.tensor.matmul(red_ps_q[:], ones_col[:], Q_acc[:], start=True, stop=True)
            cc_sb = small.tile([1, 2 * D], fp32)
            nc.scalar.copy(cc_sb[0:1, 0:D], red_ps_s[:])
            nc.scalar.copy(cc_sb[0:1, D : 2 * D], red_ps_q[:])

            cc_in = dram.tile([1, 2 * D], fp32)
            cc_out = dram.tile([1, 2 * D], fp32)
            nc.gpsimd.dma_start(cc_in[:], cc_sb[:])
            nc.gpsimd.collective_compute(
                "AllReduce",
                ALU.add,
                replica_groups=[list(range(n_cores))],
                ins=[cc_in.opt()],
                outs=[cc_out.opt()],
            )
            nc.gpsimd.dma_start(cc_sb[:], cc_out[:])
            Sg = cc_sb[0:1, 0:D]
            Qg = cc_sb[0:1, D : 2 * D]

            def scr():
                return small.tile([1, D], fp32, name="scr", tag="scr", bufs=3)

            mean = small.tile([1, D], fp32, name="mean", tag="mean")
            phi = small.tile([1, D], fp32, name="phi", tag="phi")
            negk = small.tile([1, D], fp32, name="negk", tag="negk")
            # mean = (2/R)*S  (S summed h/2)
            nc.vector.tensor_scalar_mul(mean[:], Sg, 2.0 / r_tot)
            vpe = scr()  # E[h^2] -> var -> var+eps (in place)
            nc.vector.tensor_scalar_mul(vpe[:], Qg, 1.0 / r_tot)
            m2 = scr()
            nc.vector.tensor_tensor(m2[:], mean[:], mean[:], ALU.mult)
            nc.vector.tensor_tensor(vpe[:], vpe[:], m2[:], ALU.subtract)
            nc.vector.tensor_scalar_add(vpe[:], vpe[:], BN_EPS)
            # y = sqrt(vpe) via ACT sqrt + 2 Newton steps (ACT sqrt is loose-ULP)
            y = scr()
            nc.scalar.activation(y[:], vpe[:], AF.Sqrt)
            for _ in range(2):
                r = scr()
                nc.vector.reciprocal(r[:], y[:])
                nc.vector.tensor_tensor(r[:], vpe[:], r[:], ALU.mult)
                nc.vector.tensor_tensor(y[:], y[:], r[:], ALU.add)
                nc.vector.tensor_scalar_mul(y[:], y[:], 0.5)
            # A = gamma/std ; invA = 1/A
            A = scr()
            nc.vector.reciprocal(A[:], y[:])
            nc.vector.tensor_tensor(A[:], A[:], gamma_sb[:], ALU.mult)
            invA = scr()
            nc.vector.reciprocal(invA[:], A[:])
            # phi = mean - (beta - 0.5)*invA ; negk = mean - beta*invA
            nc.vector.tensor_scalar_sub(phi[:], beta_sb[:], V_TH)
            nc.vector.tensor_tensor(phi[:], phi[:], invA[:], ALU.mult)
            nc.vector.tensor_tensor(phi[:], mean[:], phi[:], ALU.subtract)
            nc.vector.tensor_tensor(negk[:], beta_sb[:], invA[:], ALU.mult)
            nc.vector.tensor_tensor(negk[:], mean[:], negk[:], ALU.subtract)

            # broadcast phi/negk across 128 partitions via K=1 matmul
            phi_b = persist.tile([P, D], fp32)
            negk_b = persist.tile([P, D], fp32)
            bc_ps = ps_misc.tile([P, D], fp32, tag="bcps")
            nc.tensor.matmul(bc_ps[:], ones_row[:], phi[:], start=True, stop=True)
            nc.scalar.copy(phi_b[:], bc_ps[:])
            bc_ps2 = ps_misc.tile([P, D], fp32, tag="bcps")
            nc.tensor.matmul(bc_ps2[:], ones_row[:], negk[:], start=True, stop=True)
            nc.scalar.copy(negk_b[:], bc_ps2[:])

            # ---------- phase 2: LIF + mask ----------
            for b in range(B_loc):
                for ntl in range(NT):
                    n0 = ntl * P
                    z_prev = None
                    for t in range(T):
                        tid = (t * B_loc + b) * NT + ntl
                        h2 = h2_store[:, tid * D : (tid + 1) * D]
                        x_in = x_pool.tile([P, D], fp32)
                        nc.sync.dma_start(x_in[:], x_d[t, b, n0 : n0 + P, :])
                        z = z_pool.tile([P, D], fp32)
                        src = negk_b if z_prev is None else z_prev
                        # z = 0.5*z_prev + h2   (one fused DVE op)
                        nc.vector.scalar_tensor_tensor(
                            z[:], src[:], 0.5, h2, ALU.mult, ALU.add
                        )
                        s = s_pool.tile([P, D], fp32)
                        nc.vector.tensor_tensor(s[:], z[:], phi_b[:], ALU.is_ge)
                        out_t = out_pool.tile([P, D], fp32)
                        nc.gpsimd.tensor_tensor(out_t[:], x_in[:], s[:], ALU.mult)
                        # hard reset: z <- negk where spiked (mask must be int
                        # dtype on HW; 1.0f bitcast to int32 is nonzero)
                        nc.vector.copy_predicated(
                            z[:], s[:].bitcast(mybir.dt.int32), negk_b[:]
                        )
                        nc.sync.dma_start(out_d[t, b, n0 : n0 + P, :], out_t[:])
                        z_prev = z

    if not sim:
        nc.compile()
    return nc


def _get_nc():
    key = (T_FULL, B_FULL // N_CORES, N_FULL, D_FULL, N_CORES)
    if key not in _BUILD_CACHE:
        _BUILD_CACHE[key] = build_kernel(
            T_FULL,
            B_FULL // N_CORES,
            N_FULL,
            D_FULL,
            N_CORES,
            r_tot=float(T_FULL * B_FULL * N_FULL),
        )
    return _BUILD_CACHE[key]


def kernel(x, mx, W, gamma, beta, _trace=False):
    from concourse import bass_utils

    x = np.ascontiguousarray(np.asarray(x, dtype=np.float32))
    mx = np.ascontiguousarray(np.asarray(mx, dtype=np.float32))
    W = np.ascontiguousarray(np.asarray(W, dtype=np.float32))
    gamma = np.ascontiguousarray(np.asarray(gamma, dtype=np.float32))
    beta = np.ascontiguousarray(np.asarray(beta, dtype=np.float32))

    nc = _get_nc()
    b_loc = B_FULL // N_CORES
    in_maps = []
    for c in range(N_CORES):
        bs = slice(c * b_loc, (c + 1) * b_loc)
        in_maps.append(
            {
                "mx": np.ascontiguousarray(mx[:, bs]),
                "x": np.ascontiguousarray(x[:, bs]),
                "W": W,
                "gamma": gamma.reshape(1, -1),
                "beta": beta.reshape(1, -1),
            }
        )
    res = bass_utils.run_bass_kernel_spmd(
        nc, in_maps, list(range(N_CORES)), trace=_trace
    )
    out = np.concatenate([res.results[c]["out"] for c in range(N_CORES)], axis=1)
    if _trace:
        kernel._last_result = res
    return out


# revision 25
# speedup vs baseline: 1.0089x; 1.0089x over previous
"""Trainium2 Bass kernel for nn_Consolidation_111669149787.

Computation (reference):
  mxt = transpose(mx, (2,1,0,3))          # [N,B,T,D] -> [T,B,N,D]
  h   = mxt @ W.T                          # einsum tbnd,ed->tbne
  hn  = BatchNorm1d over (T*B*N) per channel e (biased var, training mode)
  s   = MultiStepLIF(hn): v <- v + (x-v)/2; s = H(v-0.5); v <- v*(1-s)
  out = x * s

Sharding: data-parallel over B (8 cores x B_loc=8); W/gamma/beta replicated.
BN stats are all-reduced across cores (1KB collective).

Key kernel trick: fold BN into the LIF recurrence. With A = gamma*invstd,
C = beta - mean*A (so hn = A*h + C) and state z = v/A - C/A, the recursion
becomes stats-free:  z_t = z_{t-1}/2 + h_t/2 ,  spike s = H(z - phi) with
phi = (0.5 - C)/A + ... (phi = (0.5-beta)/A + mean), reset z <- -C/A = negk.
So phase 1 (matmul + stat partials) runs before stats are known, and phase 2
is 3 DVE ops + 1 GpSimd op per [128,512] tile-step.
"""

import sys

sys.path.insert(0, "/opt/trn_rl_repo")

import numpy as np

# Full-problem constants (hardcoded per contract: kernel.py is self-contained)
T_FULL, B_FULL, N_FULL, D_FULL = 4, 64, 256, 512
N_CORES = 8
TAU_HALF = 0.5  # 1/tau with tau=2.0
V_TH = 0.5
BN_EPS = 1e-5

_BUILD_CACHE = {}


def build_kernel(T, B_loc, N, D, n_cores, r_tot, sim=False, mm_n=256):
    """Build the SPMD Bass program for one core's shard.

    Per-core inputs: mx [N, B_loc, T, D], x [T, B_loc, N, D], WT [D, D]
    (= W.T, [d, e] layout), gamma [1, D], beta [1, D].
    Output: out [T, B_loc, N, D].
    r_tot = T * (B_loc*n_cores) * N  (global BN sample count).

    LIF state is kept in "w-form": with A = gamma*invstd, C = beta - mean*A
    (hn = A*h + C), z = v/A - C/A obeys z_t = z_{t-1}/2 + h_t/2 and
    w_t = 2^t * z_t obeys  w_t = w_{t-1} + 2^{t-1} h_t  -- a plain add.
    Spike: s_t = H(w_t - 2^t*phi), reset: w <- 2^t*negk, with
    phi = mean - (beta-0.5)/A, negk = mean - beta/A.
    """
    import concourse.bass as bass
    import concourse.bacc as bacc
    import concourse.mybir as mybir
    import concourse.tile as tile
    from concourse import masks

    fp32 = mybir.dt.float32
    P = 128
    assert D % P == 0 and N % P == 0
    DB = D // P  # d-blocks (4)
    NT = N // P  # n-tiles per (t, b) (2)
    n_tiles = T * B_loc * NT  # 64 phase-1 tiles per core
    ALU = mybir.AluOpType
    AF = mybir.ActivationFunctionType

    if sim:
        nc = bacc.Bacc(None, num_devices=n_cores, target_bir_lowering=False, debug=True)
    else:
        nc = bacc.Bacc(None, num_devices=n_cores)

    mx_d = nc.declare_dram_parameter("mx", [N, B_loc, T, D], fp32, isOutput=False)
    x_d = nc.declare_dram_parameter("x", [T, B_loc, N, D], fp32, isOutput=False)
    wt_d = nc.declare_dram_parameter("WT", [D, D], fp32, isOutput=False)
    gamma_d = nc.declare_dram_parameter("gamma", [1, D], fp32, isOutput=False)
    beta_d = nc.declare_dram_parameter("beta", [1, D], fp32, isOutput=False)
    out_d = nc.declare_dram_parameter("out", [T, B_loc, N, D], fp32, isOutput=True)

    with tile.TileContext(nc) as tc:
        with (
            tc.tile_pool(name="persist", bufs=1) as persist,
            tc.tile_pool(name="mx_in", bufs=2) as mx_pool,
            tc.tile_pool(name="mxT", bufs=2) as mxT_pool,
            tc.tile_pool(name="sq", bufs=2) as sq_pool,
            tc.tile_pool(name="x_in", bufs=2) as x_pool,
            tc.tile_pool(name="out_st", bufs=2) as out_pool,
            tc.tile_pool(name="spk", bufs=2) as s_pool,
            tc.tile_pool(name="zst", bufs=2) as z_pool,
            tc.tile_pool(name="small", bufs=1) as small,
            tc.tile_pool(name="ps_tr", bufs=2, space="PSUM") as ps_tr,
            tc.tile_pool(name="ps_h", bufs=2, space="PSUM") as ps_h,
            tc.tile_pool(name="ps_misc", bufs=1, space="PSUM") as ps_misc,
            tc.tile_pool(name="dram", bufs=2, space="DRAM") as dram,
        ):
            # ---------- setup ----------
            identity = persist.tile([P, P], fp32)
            masks.make_identity(nc, identity[:])
            ones_col = persist.tile([P, 1], fp32)  # lhsT for partition reduce
            nc.gpsimd.memset(ones_col[:], 1.0)
            ones_row = persist.tile([1, P], fp32)  # lhsT for partition broadcast
            nc.gpsimd.memset(ones_row[:], 1.0)

            gamma_sb = small.tile([1, D], fp32)
            beta_sb = small.tile([1, D], fp32)
            nc.sync.dma_start(gamma_sb[:], gamma_d[:])
            nc.sync.dma_start(beta_sb[:], beta_d[:])

            # WT passed pre-transposed from host: WT[db] = [128 d-in-block, D e]
            WT = [
                persist.tile([P, D], fp32, name=f"WT{db}", tag=f"WT{db}")
                for db in range(DB)
            ]
            for db in range(DB):
                nc.sync.dma_start(WT[db][:], wt_d[db * P : (db + 1) * P, :])

            # stat partial accumulators
            S_acc = persist.tile([P, D], fp32)  # sum of h
            Q_acc = persist.tile([P, D], fp32)  # sum of h^2
            nc.vector.memset(S_acc[:], 0.0)
            nc.vector.memset(Q_acc[:], 0.0)

            # big h/2 store: one [128, n_tiles*D] fp32 tile
            h2_store = persist.tile([P, n_tiles * D], fp32)

            # ---------- phase 1: matmul + stat partials ----------
            for t in range(T):
                for b in range(B_loc):
                    for ntl in range(NT):
                        tid = (t * B_loc + b) * NT + ntl
                        n0 = ntl * P
                        mx_in = mx_pool.tile([P, D], fp32)
                        nc.sync.dma_start(
                            mx_in[:], mx_d[n0 : n0 + P, b, t, :]
                        )
                        tr_ps = ps_tr.tile([P, D], fp32, tag="trps")
                        for db in range(DB):
                            nc.tensor.transpose(
                                tr_ps[:, db * P : (db + 1) * P],
                                mx_in[:, db * P : (db + 1) * P],
                                identity[:],
                            )
                        mxT = mxT_pool.tile([P, D], fp32)
                        nc.scalar.copy(mxT[:], tr_ps[:])
                        n_chunks = D // mm_n
                        h2 = h2_store[:, tid * D : (tid + 1) * D]
                        sq = sq_pool.tile([P, D], fp32)
                        for ch in range(n_chunks):
                            cs = slice(ch * mm_n, (ch + 1) * mm_n)
                            h_ps = ps_h.tile(
                                [P, mm_n], fp32, name="h_ps", tag="h_ps",
                                bufs=2 * n_chunks,
                            )
                            for db in range(DB):
                                nc.tensor.matmul(
                                    h_ps[:],
                                    mxT[:, db * P : (db + 1) * P],
                                    WT[db][:, cs],
                                    start=(db == 0),
                                    stop=(db == DB - 1),
                                )
                            # store 2^(t-1)-scaled h for the w-form LIF
                            nc.scalar.activation(
                                h2[:, cs], h_ps[:], AF.Copy, scale=float(2**t)
                            )
                            nc.scalar.activation(sq[:, cs], h_ps[:], AF.Square)
                            nc.vector.tensor_tensor(
                                S_acc[:, cs], S_acc[:, cs], h_ps[:], ALU.add
                            )
                            nc.vector.tensor_tensor(
                                Q_acc[:, cs], Q_acc[:, cs], sq[:, cs], ALU.add
                            )

            # ---------- phase 1.5: stats + collective + fold ----------
            red_ps_s = ps_misc.tile([1, D], fp32, name="red_ps_s", tag="redps")
            red_ps_q = ps_misc.tile([1, D], fp32, name="red_ps_q", tag="redps")
            nc.tensor.matmul(red_ps_s[:], ones_col[:], S_acc[:], start=True, stop=True)
            nc.tensor.matmul(red_ps_q[:], ones_col[:], Q_acc[:], start=True, stop=True)
            cc_sb = small.tile([1, 2 * D], fp32)
            nc.scalar.copy(cc_sb[0:1, 0:D], red_ps_s[:])
            nc.scalar.copy(cc_sb[0:1, D : 2 * D], red_ps_q[:])

            cc_in = dram.tile([1, 2 * D], fp32)
            cc_out = dram.tile([1, 2 * D], fp32)
            nc.gpsimd.dma_start(cc_in[:], cc_sb[:])
            nc.gpsimd.collective_compute(
                "AllReduce",
                ALU.add,
                replica_groups=[list(range(n_cores))],
                ins=[cc_in.opt()],
                outs=[cc_out.opt()],
            )
            nc.gpsimd.dma_start(cc_sb[:], cc_out[:])
            Sg = cc_sb[0:1, 0:D]
            Qg = cc_sb[0:1, D : 2 * D]

            def scr():
                return small.tile([1, D], fp32, name="scr", tag="scr", bufs=3)

            mean = small.tile([1, D], fp32, name="mean", tag="mean")
            phi = small.tile([1, D], fp32, name="phi", tag="phi")
            negk = small.tile([1, D], fp32, name="negk", tag="negk")
            nc.vector.tensor_scalar_mul(mean[:], Sg, 1.0 / r_tot)
            vpe = scr()  # E[h^2] -> var -> var+eps (in place)
            nc.vector.tensor_scalar_mul(vpe[:], Qg, 1.0 / r_tot)
            m2 = scr()
            nc.vector.tensor_tensor(m2[:], mean[:], mean[:], ALU.mult)
            nc.vector.tensor_tensor(vpe[:], vpe[:], m2[:], ALU.subtract)
            nc.vector.tensor_scalar_add(vpe[:], vpe[:], BN_EPS)
            # invstd = rsqrt(vpe): seed y0 = ACT_sqrt(1/vpe), then two
            # multiply-only Newton steps  y <- y*(1.5 - 0.5*vpe*y^2)
            rvpe = scr()
            nc.vector.reciprocal(rvpe[:], vpe[:])
            y = scr()
            nc.scalar.activation(y[:], rvpe[:], AF.Sqrt)
            for _ in range(2):
                t_ = scr()
                nc.vector.tensor_tensor(t_[:], y[:], y[:], ALU.mult)
                nc.vector.tensor_tensor(t_[:], vpe[:], t_[:], ALU.mult)
                nc.vector.tensor_scalar(t_[:], t_[:], -0.5, 1.5, ALU.mult, ALU.add)
                nc.vector.tensor_tensor(y[:], y[:], t_[:], ALU.mult)
            # A = gamma*invstd ; invA = 1/A
            A = scr()
            nc.vector.tensor_tensor(A[:], y[:], gamma_sb[:], ALU.mult)
            invA = scr()
            nc.vector.reciprocal(invA[:], A[:])
            # phi = mean - (beta - 0.5)*invA ; negk = mean - beta*invA
            nc.vector.tensor_scalar_sub(phi[:], beta_sb[:], V_TH)
            nc.vector.tensor_tensor(phi[:], phi[:], invA[:], ALU.mult)
            nc.vector.tensor_tensor(phi[:], mean[:], phi[:], ALU.subtract)
            nc.vector.tensor_tensor(negk[:], beta_sb[:], invA[:], ALU.mult)
            nc.vector.tensor_tensor(negk[:], mean[:], negk[:], ALU.subtract)

            # Broadcast across 128 partitions via K=1 matmuls:
            # negk (w_0 init) plus per-timestep 2^(t+1)-scaled phi and negk.
            def broadcast(src_ap, name):
                dst = persist.tile([P, D], fp32, name=name, tag=name)
                bc_ps = ps_misc.tile([P, D], fp32, name=f"bc_{name}", tag="bcps")
                nc.tensor.matmul(bc_ps[:], ones_row[:], src_ap, start=True, stop=True)
                nc.scalar.copy(dst[:], bc_ps[:])
                return dst

            negk_b = broadcast(negk[:], "negk_b")
            phi_t = []
            negk_t = []
            sct = scr()
            for t in range(T):
                sc = float(2 ** (t + 1))
                nc.vector.tensor_scalar_mul(sct[:], phi[:], sc)
                phi_t.append(broadcast(sct[:], f"phi_t{t}"))
                nc.vector.tensor_scalar_mul(sct[:], negk[:], sc)
                negk_t.append(broadcast(sct[:], f"negk_t{t}"))

            # ---------- phase 2: LIF + mask (w-form) ----------
            for b in range(B_loc):
                for ntl in range(NT):
                    n0 = ntl * P
                    w_prev = None
                    for t in range(T):
                        tid = (t * B_loc + b) * NT + ntl
                        h2 = h2_store[:, tid * D : (tid + 1) * D]
                        x_in = x_pool.tile([P, D], fp32)
                        nc.sync.dma_start(x_in[:], x_d[t, b, n0 : n0 + P, :])
                        w = z_pool.tile([P, D], fp32, name="w", tag="w")
                        src = negk_b if w_prev is None else w_prev
                        nc.vector.tensor_tensor(w[:], src[:], h2, ALU.add)
                        s = s_pool.tile([P, D], fp32)
                        nc.vector.tensor_tensor(s[:], w[:], phi_t[t][:], ALU.is_ge)
                        out_t = out_pool.tile([P, D], fp32)
                        nc.gpsimd.tensor_tensor(out_t[:], x_in[:], s[:], ALU.mult)
                        # hard reset: w <- 2^(t+1)*negk where spiked (mask must
                        # be int dtype on HW; 1.0f bitcast to int32 is nonzero)
                        nc.vector.copy_predicated(
                            w[:], s[:].bitcast(mybir.dt.int32), negk_t[t][:]
                        )
                        nc.sync.dma_start(out_d[t, b, n0 : n0 + P, :], out_t[:])
                        w_prev = w

    if not sim:
        nc.compile()
    return nc


def _get_nc():
    key = (T_FULL, B_FULL // N_CORES, N_FULL, D_FULL, N_CORES)
    if key not in _BUILD_CACHE:
        _BUILD_CACHE[key] = build_kernel(
            T_FULL,
            B_FULL // N_CORES,
            N_FULL,
            D_FULL,
            N_CORES,
            r_tot=float(T_FULL * B_FULL * N_FULL),
        )
    return _BUILD_CACHE[key]


def kernel(x, mx, W, gamma, beta, _trace=False):
    from concourse import bass_utils

    x = np.ascontiguousarray(np.asarray(x, dtype=np.float32))
    mx = np.ascontiguousarray(np.asarray(mx, dtype=np.float32))
    W = np.ascontiguousarray(np.asarray(W, dtype=np.float32))
    gamma = np.ascontiguousarray(np.asarray(gamma, dtype=np.float32))
    beta = np.ascontiguousarray(np.asarray(beta, dtype=np.float32))

    nc = _get_nc()
    b_loc = B_FULL // N_CORES
    wt = np.ascontiguousarray(W.T)
    in_maps = []
    for c in range(N_CORES):
        bs = slice(c * b_loc, (c + 1) * b_loc)
        in_maps.append(
            {
                "mx": np.ascontiguousarray(mx[:, bs]),
                "x": np.ascontiguousarray(x[:, bs]),
                "WT": wt,
                "gamma": gamma.reshape(1, -1),
                "beta": beta.reshape(1, -1),
            }
        )
    res = bass_utils.run_bass_kernel_spmd(
        nc, in_maps, list(range(N_CORES)), trace=_trace
    )
    out = np.concatenate([res.results[c]["out"] for c in range(N_CORES)], axis=1)
    if _trace:
        kernel._last_result = res
    return out
